# revision 1
# baseline (speedup 1.0000x reference)
"""Trainium2 Bass kernel for nn_Actor_GNN (GATv2 x2 + MLP actor head).

Sharding: data-parallel over the 1024 independent graphs -> 128 graphs/core
on 8 NeuronCores.  Per core: 4096 nodes, 32768 edges, edges of graph b are
the contiguous block [b*256,(b+1)*256) and reference nodes [b*32,(b+1)*32).

Strategy: gather/segment-softmax/scatter are expressed as dense matmuls with
one-hot src/dst selection matrices (built host-side, bf16):
  pre[e,:] = S@xl + D@xr + ea@We.T         (TensorE, PSUM accumulate)
  score    = sum_c leakyrelu(pre)*att      (linear part folded into matmul
                                            columns; only att*min(pre,0)
                                            computed on VectorE)
  A_T'[m,(h,n)] = sum_e S[e,m] ex[e,h] D[e,n]   (per-graph 32x32 attention)
  den[n,h] = sum_e D[e,n] ex[e,h]
  out[n,:] = (A_T'_h.T @ xl_h) * (1/den) + bias  (normalize after aggregation)
"""

import sys

sys.path.insert(0, "/opt/trn_rl_repo")

import numpy as np
import ml_dtypes

import concourse.bass as bass
import concourse.mybir as mybir
from concourse import bacc
from concourse import tile
from concourse.bass_utils import run_bass_kernel_spmd
from concourse.masks import make_identity

BF = mybir.dt.bfloat16
F32 = mybir.dt.float32
AF = mybir.ActivationFunctionType
ALU = mybir.AluOpType
bf16 = ml_dtypes.bfloat16

# ---- problem constants (hardcoded) ----
B, NG, F, ED, EG = 1024, 32, 16, 6, 256
H1, C1, C2 = 5, 80, 160
OBS, ACT = 512, 2
NCORES = 8
G = B // NCORES          # 128 graphs per core
NNODES = G * NG          # 4096
NEDGES = G * EG          # 32768
NGRP = NNODES // 128     # 32 groups of 128 nodes (4 graphs)
ETPG = 8                 # edge tiles (of 128) per group (1024 edges)
HC1 = H1 * C1            # 400

_CACHE = {}


def build_nc():
    nc = bacc.Bacc("TRN2", target_bir_lowering=False, debug=False)

    def par(name, shape, dt, out=False):
        return nc.declare_dram_parameter(name, list(shape), dt, isOutput=out)

    d_xT = par("xT", [17, NNODES], BF)            # x.T with ones row 16
    d_ea = par("ea6", [ED, NEDGES], BF)           # edge_attr.T
    d_ST = par("ST", [128, NEDGES], BF)           # src one-hot, node-in-group major
    d_DT = par("DT", [128, NEDGES], BF)           # dst one-hot
    d_Se = par("SeR", [NGRP, 128, ETPG, NG], BF)   # src one-hot edge-major
    d_De = par("DeR", [NGRP, 128, ETPG, NG], BF)
    d_W1 = par("Wlr1b", [17, 820], BF)            # [Wl1.T|a] , [Wr1.T|a] (+bias row)
    d_We1 = par("We1be", [ED, 410], BF)           # [We1.T | We1.T@att]
    d_att1 = par("att1rep", [128, 410], BF)       # att1 flat, replicated
    d_bc1 = par("bc1rep", [128, HC1], F32)
    d_W2 = par("Wlr2b", [128, 4, 324], BF)        # chunked [512pad, 322]
    d_We2 = par("We2be", [ED, 162], BF)
    d_att2 = par("att2rep", [128, 162], BF)
    d_bc2 = par("bc2rep", [128, C2], F32)
    d_wd1 = par("Wd1T", [C2, 32], BF)
    d_wd2 = par("Wd2T", [32, OBS], BF)
    d_wf1 = par("Wf1T", [128, 4, 256], BF)        # [512,256] chunked on k
    d_wf2 = par("Wf2T", [128, 2, 256], BF)
    d_wms = par("WmsT", [128, 2, 4], BF)          # [256, 4] chunked
    d_bd1 = par("bd1", [32, 1], F32)
    d_bd2 = par("bd2c", [128, 4], F32)
    d_bf1 = par("bf1c", [128, 2], F32)
    d_bf2 = par("bf2c", [128, 2], F32)
    d_bms = par("bms", [4, 1], F32)
    d_out = par("out", [4, 128], F32, out=True)

    with tile.TileContext(nc) as tc:
        import contextlib

        ctx = contextlib.ExitStack()
        with ctx:
            cpool = ctx.enter_context(tc.tile_pool(name="const", bufs=1))
            persist = ctx.enter_context(tc.tile_pool(name="persist", bufs=1))
            spool = ctx.enter_context(tc.tile_pool(name="stream", bufs=5))
            wpool = ctx.enter_context(tc.tile_pool(name="work", bufs=5))
            npool = ctx.enter_context(tc.tile_pool(name="small", bufs=5))
            ppool = ctx.enter_context(tc.tile_pool(name="psum", bufs=1, space="PSUM"))
            ppre = ctx.enter_context(tc.tile_pool(name="psumpre", bufs=5, space="PSUM"))
            ppool1 = ctx.enter_context(tc.tile_pool(name="psum1", bufs=2, space="PSUM"))

            # ---- constants to SBUF ----
            def cload(dram, shape, dt):
                t = cpool.tile(list(shape), dt, tag=dram.name + "_c")
                nc.sync.dma_start(out=t[...], in_=dram.ap())
                return t

            s_W1 = cload(d_W1, [17, 820], BF)
            s_We1 = cload(d_We1, [ED, 410], BF)
            s_att1 = cload(d_att1, [128, 410], BF)
            s_bc1 = cload(d_bc1, [128, HC1], F32)
            s_W2 = cload(d_W2, [128, 4, 324], BF)
            s_We2 = cload(d_We2, [ED, 162], BF)
            s_att2 = cload(d_att2, [128, 162], BF)
            s_bc2 = cload(d_bc2, [128, C2], F32)
            s_wd1a = cpool.tile([128, 32], BF, tag="wd1a")
            nc.sync.dma_start(out=s_wd1a[...], in_=d_wd1.ap()[0:128, :])
            s_wd1b = cpool.tile([32, 32], BF, tag="wd1b")
            nc.sync.dma_start(out=s_wd1b[...], in_=d_wd1.ap()[128:160, :])
            s_wd2 = cload(d_wd2, [32, OBS], BF)
            s_wf1 = cload(d_wf1, [128, 4, 256], BF)
            s_wf2 = cload(d_wf2, [128, 2, 256], BF)
            s_wms = cload(d_wms, [128, 2, 4], BF)
            s_bd1 = cload(d_bd1, [32, 1], F32)
            s_bd2 = cload(d_bd2, [128, 4], F32)
            s_bf1 = cload(d_bf1, [128, 2], F32)
            s_bf2 = cload(d_bf2, [128, 2], F32)
            s_bm = cpool.tile([2, 1], F32, tag="bm_c")
            nc.sync.dma_start(out=s_bm[...], in_=d_bms.ap()[0:2, :])
            s_bs = cpool.tile([2, 1], F32, tag="bs_c")
            nc.sync.dma_start(out=s_bs[...], in_=d_bms.ap()[2:4, :])

            ident = cpool.tile([128, 128], BF, tag="ident")
            make_identity(nc, ident[...])

            # ---- persistent activations ----
            xlr = persist.tile([128, NGRP, 820], BF)    # [xl|a1l|xr|a1r] per group
            h1 = persist.tile([128, NGRP, HC1], BF)
            h1T = persist.tile([128, 4, NNODES], BF)    # h1 transposed, c-chunked
            xlr2 = persist.tile([128, NGRP, 324], BF)
            h2 = persist.tile([128, NGRP, C2], BF)
            ego = persist.tile([128, C2], BF)
            egoT0 = persist.tile([128, 128], BF)
            egoT1 = persist.tile([32, 128], BF)
            d1_sb = persist.tile([32, 128], BF)
            d_sb = persist.tile([128, 4, 128], BF)
            f1_sb = persist.tile([128, 2, 128], BF)
            f2_sb = persist.tile([128, 2, 128], BF)
            out_m = persist.tile([2, 128], F32)
            out_s = persist.tile([2, 128], F32)
            ts_sb = persist.tile([2, 128], F32)

            # h1T padding rows: zero tail of chunk 3, ones row 400 (=chunk3 row 16)
            nc.gpsimd.memset(h1T[:, 3, :], 0.0)
            nc.vector.memset(h1T[32:64, 3, :], 1.0)

            # ---- per-group main loop: 3-deep software pipeline ----
            # cycle g emits: l1rest(g) | s1(g+2) tiles 0-3 | ladder(g) |
            #                s1(g+2) tiles 4-7 interleaved with l2pre(g) | l2rest(g)
            def s1_dma(grp):
                # layer-1 node transform for this group (phase A, pipelined)
                xt = spool.tile([17, 128], BF, tag="xt")
                nc.sync.dma_start(out=xt[...], in_=d_xT.ap()[:, grp * 128:(grp + 1) * 128])
                psA = ppre.tile([128, 410], F32, tag="pre")
                psB = ppre.tile([128, 410], F32, tag="pre")
                nc.tensor.matmul(psA[...], xt[...], s_W1[:, 0:410], start=True, stop=True)
                nc.tensor.matmul(psB[...], xt[...], s_W1[:, 410:820], start=True, stop=True)
                nc.scalar.activation(xlr[:, grp, 0:410], psA[...], AF.Copy)
                nc.vector.tensor_copy(xlr[:, grp, 410:820], psB[...])

                e0 = grp * 1024
                st_t = spool.tile([128, 1024], BF, tag="st")
                dt_t = spool.tile([128, 1024], BF, tag="dt")
                se_t = spool.tile([128, ETPG, NG], BF, tag="se")
                de_t = spool.tile([128, ETPG, NG], BF, tag="de")
                ea1 = spool.tile([ED, 1024], BF, tag="ea")
                nc.sync.dma_start(out=st_t[...], in_=d_ST.ap()[:, e0:e0 + 1024])
                nc.sync.dma_start(out=dt_t[...], in_=d_DT.ap()[:, e0:e0 + 1024])
                nc.sync.dma_start(out=se_t[...], in_=d_Se.ap()[grp])
                nc.sync.dma_start(out=de_t[...], in_=d_De.ap()[grp])
                nc.sync.dma_start(out=ea1[...], in_=d_ea.ap()[:, e0:e0 + 1024])
                score1 = npool.tile([128, ETPG, H1], F32, tag="score", bufs=6)
                return dict(st=st_t, dt=dt_t, se=se_t, de=de_t, ea=ea1, score1=score1)

            def s1_tile(grp, S, t):
                pre = ppre.tile([128, 410], F32, tag="pre")
                nc.tensor.matmul(pre[...], S["st"][:, t * 128:(t + 1) * 128],
                                 xlr[:, grp, 0:410], start=True, stop=False)
                nc.tensor.matmul(pre[...], S["dt"][:, t * 128:(t + 1) * 128],
                                 xlr[:, grp, 410:820], start=False, stop=False)
                nc.tensor.matmul(pre[...], S["ea"][:, t * 128:(t + 1) * 128],
                                 s_We1[...], start=False, stop=True)
                pma = wpool.tile([128, 410], BF, tag="pma")
                nc.vector.scalar_tensor_tensor(pma[...], pre[:, 0:410], 0.0, s_att1[...],
                                               op0=ALU.max, op1=ALU.mult)
                nc.vector.tensor_reduce(S["score1"][:, t, :],
                                        pma[...].rearrange("p (h c) -> p h c", h=H1),
                                        axis=mybir.AxisListType.X, op=ALU.add)

            def l1rest(grp, S):
                se_t, de_t = S["se"], S["de"]
                ex1 = npool.tile([128, ETPG, H1], BF, tag="ex")
                nc.scalar.activation(ex1[...], S["score1"][...], AF.Exp)
                at1 = ppool1.tile([128, H1 * NG], F32, tag="ats", bufs=1)
                for t in range(ETPG):
                    g = t // 2
                    first = (t % 2 == 0)
                    exd = wpool.tile([128, H1, NG], BF, tag="exd", bufs=8)
                    nc.vector.scalar_tensor_tensor(
                        exd[...],
                        de_t[:, t, None, :].broadcast_to([128, H1, NG]),
                        0.0,
                        ex1[:, t, :, None].broadcast_to([128, H1, NG]),
                        op0=ALU.bypass, op1=ALU.mult)
                    nc.tensor.matmul(at1[32 * g:32 * (g + 1), :], se_t[:, t, :],
                                     exd[...], start=first, stop=not first,
                                     tile_position=(0, 32 * g))
                at1_sb = wpool.tile([128, H1 * NG], BF, tag="at1sb")
                nc.scalar.activation(at1_sb[...], at1[...], AF.Copy)
                o1 = ppool.tile([128, H1, 81], F32, tag="agg")
                for g in range(4):
                    for h in range(H1):
                        nc.tensor.matmul(
                            o1[32 * g:32 * (g + 1), h, :],
                            at1_sb[32 * g:32 * (g + 1), h * 32:(h + 1) * 32],
                            xlr[32 * g:32 * (g + 1), grp, h * 82:h * 82 + 81],
                            start=True, stop=True, tile_position=(32 * g, 32 * g))
                den1 = npool.tile([128, H1], F32, tag="densb")
                nc.vector.tensor_scalar_add(den1[...], o1[:, :, 80], 1e-16)
                rec1 = npool.tile([128, H1], F32, tag="rec")
                nc.vector.reciprocal(rec1[...], den1[...])
                t1 = wpool.tile([128, HC1], BF, tag="t1")
                for h in range(H1):
                    hs = slice(h * C1, (h + 1) * C1)
                    nc.vector.scalar_tensor_tensor(t1[:, hs], o1[:, h, 0:80], rec1[:, h:h + 1],
                                                   s_bc1[:, hs], op0=ALU.mult, op1=ALU.add)
                nc.scalar.activation(h1[:, grp, :], t1[...], AF.Relu)

            def ladder(grp):
                trp = ppool.tile([128, 4, 128], BF, tag="agg")
                for j in range(4):
                    csz = 128 if j < 3 else 16
                    nc.tensor.transpose(trp[0:csz, j, :], h1[:, grp, j * 128:j * 128 + csz],
                                        ident[...])
                for j in range(4):
                    csz = 128 if j < 3 else 16
                    nc.vector.tensor_copy(h1T[0:csz, j, grp * 128:(grp + 1) * 128],
                                          trp[0:csz, j, :])
            def xlr2fn(grp):
                ps2 = ppre.tile([128, 324], F32, tag="ps2", bufs=1)
                for j in range(4):
                    nc.tensor.matmul(ps2[...], h1T[:, j, grp * 128:(grp + 1) * 128],
                                     s_W2[:, j, :], start=(j == 0), stop=(j == 3))
                nc.scalar.activation(xlr2[:, grp, :], ps2[...], AF.Copy)

            def l2pre_tile(grp, S, t):
                pre2 = ppre.tile([128, 162], F32, tag="pre")
                nc.tensor.matmul(pre2[...], S["st"][:, t * 128:(t + 1) * 128],
                                 xlr2[:, grp, 0:162], start=True, stop=False)
                nc.tensor.matmul(pre2[...], S["dt"][:, t * 128:(t + 1) * 128],
                                 xlr2[:, grp, 162:324], start=False, stop=False)
                nc.tensor.matmul(pre2[...], S["ea"][:, t * 128:(t + 1) * 128],
                                 s_We2[...], start=False, stop=True)
                pma2 = wpool.tile([128, 162], BF, tag="pma")
                nc.vector.scalar_tensor_tensor(pma2[...], pre2[:, 0:162], 0.0, s_att2[...],
                                               op0=ALU.max, op1=ALU.mult,
                                               accum_out=S["score2"][:, t:t + 1])

            def l2rest(grp, S):
                se_t, de_t = S["se"], S["de"]
                ex2 = npool.tile([128, ETPG], BF, tag="ex")
                nc.scalar.activation(ex2[...], S["score2"][...], AF.Exp)
                at2 = ppool1.tile([128, NG], F32, tag="ats", bufs=1)
                for t in range(ETPG):
                    g = t // 2
                    first = (t % 2 == 0)
                    exd2 = wpool.tile([128, NG], BF, tag="exd", bufs=8)
                    nc.vector.scalar_tensor_tensor(
                        exd2[...], de_t[:, t, :], 0.0,
                        ex2[:, t:t + 1].broadcast_to([128, NG]),
                        op0=ALU.bypass, op1=ALU.mult)
                    nc.tensor.matmul(at2[32 * g:32 * (g + 1), :], se_t[:, t, :],
                                     exd2[...], start=first, stop=not first,
                                     tile_position=(0, 32 * g))
                at2_sb = wpool.tile([128, NG], BF, tag="at2sb")
                nc.scalar.activation(at2_sb[...], at2[...], AF.Copy)
                o2 = ppool.tile([128, C2 + 1], F32, tag="agg")
                for g in range(4):
                    nc.tensor.matmul(o2[32 * g:32 * (g + 1), :], at2_sb[32 * g:32 * (g + 1), :],
                                     xlr2[32 * g:32 * (g + 1), grp, 0:C2 + 1],
                                     start=True, stop=True, tile_position=(32 * g, 32 * g))
                den2 = npool.tile([128, 1], F32, tag="densb")
                nc.vector.tensor_scalar_add(den2[...], o2[:, C2:C2 + 1], 1e-16)
                rec2 = npool.tile([128, 1], F32, tag="rec")
                nc.vector.reciprocal(rec2[...], den2[...])
                t2 = wpool.tile([128, C2], BF, tag="t1")
                nc.vector.scalar_tensor_tensor(t2[...], o2[:, 0:C2], rec2[:, 0:1], s_bc2[...],
                                               op0=ALU.mult, op1=ALU.add)
                nc.scalar.activation(h2[:, grp, :], t2[...], AF.Relu)
                # extract this group's 4 ego rows (graphs 4*grp + a), natural order
                nc.sync.dma_start(out=ego[4 * grp:4 * grp + 4, :],
                                  in_=h2[0:128:32, grp, :])

            Ss = {}
            for g0 in (0, 1):
                Ss[g0] = s1_dma(g0)
                for t in range(ETPG):
                    s1_tile(g0, Ss[g0], t)
            for g in range(NGRP):
                S = Ss[g]
                S["score2"] = npool.tile([128, ETPG], F32, tag="score2", name="score2")
                l1rest(g, S)
                if g + 2 < NGRP:
                    Ss[g + 2] = s1_dma(g + 2)
                    for t in range(4):
                        s1_tile(g + 2, Ss[g + 2], t)
                ladder(g)
                xlr2fn(g)
                if g + 2 < NGRP:
                    for t in range(4, ETPG):
                        s1_tile(g + 2, Ss[g + 2], t)
                for t in range(ETPG):
                    l2pre_tile(g, S, t)
                l2rest(g, S)
                del Ss[g]

            tpa = ppool.tile([128, 128], BF, tag="agg")
            nc.tensor.transpose(tpa[...], ego[:, 0:128], ident[...])
            nc.scalar.activation(egoT0[...], tpa[...], AF.Copy)
            tpb = ppool.tile([32, 128], BF, tag="agg")
            nc.tensor.transpose(tpb[...], ego[:, 128:160], ident[...])
            nc.scalar.activation(egoT1[...], tpb[...], AF.Copy)

            # ---- head MLP (feature-major, batch=128 on free dim) ----
            d1p = ppool.tile([32, 128], F32, tag="agg")
            nc.tensor.matmul(d1p[...], s_wd1a[...], egoT0[...], start=True, stop=False)
            nc.tensor.matmul(d1p[...], s_wd1b[...], egoT1[...], start=False, stop=True)
            nc.scalar.activation(d1_sb[...], d1p[...], AF.Identity, bias=s_bd1[...])
            for j in range(4):
                dp = ppool.tile([128, 128], F32, tag="agg")
                nc.tensor.matmul(dp[...], s_wd2[:, j * 128:(j + 1) * 128], d1_sb[...],
                                 start=True, stop=True)
                nc.scalar.activation(d_sb[:, j, :], dp[...], AF.Tanh, bias=s_bd2[:, j:j + 1])
            for m in range(2):
                fp = ppool.tile([128, 128], F32, tag="agg")
                for j in range(4):
                    nc.tensor.matmul(fp[...], s_wf1[:, j, m * 128:(m + 1) * 128], d_sb[:, j, :],
                                     start=(j == 0), stop=(j == 3))
                nc.scalar.activation(f1_sb[:, m, :], fp[...], AF.Relu, bias=s_bf1[:, m:m + 1])
            for m in range(2):
                fp2 = ppool.tile([128, 128], F32, tag="agg")
                for j in range(2):
                    nc.tensor.matmul(fp2[...], s_wf2[:, j, m * 128:(m + 1) * 128], f1_sb[:, j, :],
                                     start=(j == 0), stop=(j == 1))
                nc.scalar.activation(f2_sb[:, m, :], fp2[...], AF.Relu, bias=s_bf2[:, m:m + 1])
            msp = ppool.tile([2, 128], F32, tag="agg")
            ssp = ppool.tile([2, 128], F32, tag="agg")
            for j in range(2):
                nc.tensor.matmul(msp[...], s_wms[:, j, 0:2], f2_sb[:, j, :],
                                 start=(j == 0), stop=(j == 1))
            for j in range(2):
                nc.tensor.matmul(ssp[...], s_wms[:, j, 2:4], f2_sb[:, j, :],
                                 start=(j == 0), stop=(j == 1))
            nc.scalar.activation(out_m[...], msp[...], AF.Identity, bias=s_bm[...])
            nc.scalar.activation(ts_sb[...], ssp[...], AF.Tanh, bias=s_bs[...])
            nc.scalar.activation(out_s[...], ts_sb[...], AF.Copy, scale=3.5, bias=-1.5)
            nc.sync.dma_start(out=d_out.ap()[0:2, :], in_=out_m[...])
            nc.sync.dma_start(out=d_out.ap()[2:4, :], in_=out_s[...])

    nc.compile()
    return nc


def _prep_core_inputs(inputs, c):
    """Host-side preprocessing for core c (all free / untimed)."""
    ns = slice(c * NNODES, (c + 1) * NNODES)
    es = slice(c * NEDGES, (c + 1) * NEDGES)
    x = np.asarray(inputs["x"])[ns]                       # [4096, 16]
    ea = np.asarray(inputs["edge_attr"])[es]              # [32768, 6]
    src = np.asarray(inputs["edge_index"])[0, es] - c * NNODES
    dst = np.asarray(inputs["edge_index"])[1, es] - c * NNODES

    e = np.arange(NEDGES)
    ST = np.zeros((128, NEDGES), np.float32)
    DT = np.zeros((128, NEDGES), np.float32)
    ST[src % 128, e] = 1.0
    DT[dst % 128, e] = 1.0
    Se = np.zeros((NEDGES, NG), np.float32)
    De = np.zeros((NEDGES, NG), np.float32)
    Se[e, src % NG] = 1.0
    De[e, dst % NG] = 1.0
    SeR = Se.reshape(NGRP, ETPG, 128, NG).transpose(0, 2, 1, 3)
    DeR = De.reshape(NGRP, ETPG, 128, NG).transpose(0, 2, 1, 3)

    xT = np.concatenate([x.T, np.ones((1, NNODES), np.float32)], 0)  # [17, 4096]

    return {
        "xT": xT.astype(bf16),
        "ea6": ea.T.astype(bf16).copy(),
        "ST": ST.astype(bf16),
        "DT": DT.astype(bf16),
        "SeR": np.ascontiguousarray(SeR).astype(bf16),
        "DeR": np.ascontiguousarray(DeR).astype(bf16),
    }


def _prep_weights(inputs):
    M2 = 4.0      # M/2: term1 offset (cancels in softmax), keeps relu slot positive
    ii = {k: np.asarray(v).astype(np.float32) for k, v in inputs.items()
          if k not in ("x", "edge_index", "edge_attr")}
    att1 = ii["att1"]                                     # [5, 80]

    def inter1(Wt, is_bias_row):
        """Interleave [*, 400] -> [*, 410]: per head [W(80) | ones | att-dot]."""
        out = np.zeros((Wt.shape[0], 410), np.float32)
        for h in range(H1):
            out[:, h * 82:h * 82 + 80] = Wt[:, h * C1:(h + 1) * C1]
            out[:, h * 82 + 81] = Wt[:, h * C1:(h + 1) * C1] @ att1[h]
            if is_bias_row:
                out[-1, h * 82 + 80] = 1.0      # ones col (agg reads l-side only)
                out[-1, h * 82 + 81] += M2
        return out

    Wl = np.concatenate([ii["Wl1"].T, ii["bl1"][None, :]], 0)   # [17, 400]
    Wr = np.concatenate([ii["Wr1"].T, ii["br1"][None, :]], 0)
    W1 = np.zeros((17, 820), np.float32)
    W1[:, 0:410] = inter1(Wl, True)
    W1[:, 410:820] = inter1(Wr, True)
    We1 = inter1(ii["We1"].T, False)                 # [6, 410]
    attm1 = np.zeros(410, np.float32)
    for h in range(H1):
        attm1[h * 82:h * 82 + 80] = 0.8 * att1[h]
        attm1[h * 82 + 81] = 0.2

        att2 = ii["att2"].reshape(-1)                         # [160]
    W2 = np.zeros((512, 324), np.float32)
    W2[0:400, 0:160] = ii["Wl2"].T
    W2[0:400, 161] = ii["Wl2"].T @ att2
    W2[0:400, 162:322] = ii["Wr2"].T
    W2[0:400, 323] = ii["Wr2"].T @ att2
    W2[416, 0:160] = ii["bl2"]
    W2[416, 160] = 1.0                               # ones col (l side)
    W2[416, 161] = ii["bl2"] @ att2 + M2
    W2[416, 162:322] = ii["br2"]
    W2[416, 323] = ii["br2"] @ att2 + M2
    We2 = np.zeros((ED, 162), np.float32)
    We2[:, 0:160] = ii["We2"].T
    We2[:, 161] = ii["We2"].T @ att2
    attm2 = np.zeros(162, np.float32)
    attm2[0:160] = 0.8 * att2
    attm2[161] = 0.2

    Wf1 = ii["Wf1"].T.reshape(4, 128, 256).copy()         # [512,256] -> k-chunks
    Wf2 = ii["Wf2"].T.reshape(2, 128, 256).copy()
    Wms = np.concatenate([ii["Wm"].T, ii["Ws"].T], 1)     # [256, 4]
    Wmsr = Wms.reshape(2, 128, 4).copy()

    return {
        "Wlr1b": W1.astype(bf16),
        "We1be": We1.astype(bf16),
        "att1rep": np.broadcast_to(attm1, (128, 410)).astype(bf16).copy(),
        "bc1rep": np.broadcast_to(ii["bc1"], (128, 400)).astype(np.float32).copy(),
        "Wlr2b": W2.reshape(4, 128, 324).transpose(1, 0, 2).astype(bf16).copy(),
        "We2be": We2.astype(bf16),
        "att2rep": np.broadcast_to(attm2, (128, 162)).astype(bf16).copy(),
        "bc2rep": np.broadcast_to(ii["bc2"], (128, 160)).astype(np.float32).copy(),
        "Wd1T": ii["Wd1"].T.astype(bf16).copy(),
        "Wd2T": ii["Wd2"].T.astype(bf16).copy(),
        "Wf1T": np.transpose(Wf1, (1, 0, 2)).astype(bf16).copy(),
        "Wf2T": np.transpose(Wf2, (1, 0, 2)).astype(bf16).copy(),
        "WmsT": np.transpose(Wmsr, (1, 0, 2)).astype(bf16).copy(),
        "bd1": ii["bd1"][:, None].astype(np.float32).copy(),
        "bd2c": ii["bd2"].reshape(4, 128).T.astype(np.float32).copy(),
        "bf1c": ii["bf1"].reshape(2, 128).T.astype(np.float32).copy(),
        "bf2c": ii["bf2"].reshape(2, 128).T.astype(np.float32).copy(),
        "bms": np.concatenate([ii["bm"], ii["bs"]])[:, None].astype(np.float32).copy(),
    }


LAST = {}


def kernel(**inputs):
    if "nc" not in _CACHE:
        _CACHE["nc"] = build_nc()
    nc = _CACHE["nc"]

    wts = _prep_weights(inputs)
    in_maps = []
    for c in range(NCORES):
        m = dict(wts)
        m.update(_prep_core_inputs(inputs, c))
        in_maps.append(m)

    res = run_bass_kernel_spmd(nc, in_maps, core_ids=list(range(NCORES)),
                               trace=LAST.get("trace", False))
    LAST["exec_time_ns"] = res.exec_time_ns
    LAST["res"] = res

    mean = np.zeros((B, ACT), np.float32)
    logstd = np.zeros((B, ACT), np.float32)
    p = np.arange(128)
    gperm = p                             # partition p holds graph p
    for c in range(NCORES):
        o = res.results[c]["out"]          # [4, 128]
        mean[c * G + gperm, :] = o[0:2, :].T
        logstd[c * G + gperm, :] = o[2:4, :].T
    return mean, logstd



# revision 56
# speedup vs baseline: 1.6258x; 1.6258x over previous
"""Trainium2 Bass kernel for nn_Actor_GNN (GATv2 x2 + MLP actor head).

Sharding: data-parallel over the 1024 independent graphs -> 128 graphs/core
on 8 NeuronCores.  Per core: 4096 nodes, 32768 edges, edges of graph b are
the contiguous block [b*256,(b+1)*256) and reference nodes [b*32,(b+1)*32).

Strategy: gather/segment-softmax/scatter are expressed as dense matmuls with
one-hot src/dst selection matrices (built host-side, bf16):
  pre[e,:] = S@xl + D@xr + ea@We.T         (TensorE, PSUM accumulate)
  score    = sum_c leakyrelu(pre)*att      (linear part folded into matmul
                                            columns; only att*min(pre,0)
                                            computed on VectorE)
  A_T'[m,(h,n)] = sum_e S[e,m] ex[e,h] D[e,n]   (per-graph 32x32 attention)
  den[n,h] = sum_e D[e,n] ex[e,h]
  out[n,:] = (A_T'_h.T @ xl_h) * (1/den) + bias  (normalize after aggregation)
"""

import sys

sys.path.insert(0, "/opt/trn_rl_repo")

import numpy as np
import ml_dtypes

import concourse.bass as bass
import concourse.mybir as mybir
from concourse import bacc
from concourse import tile
from concourse.bass_utils import run_bass_kernel_spmd
from concourse.masks import make_identity

BF = mybir.dt.bfloat16
F32 = mybir.dt.float32
AF = mybir.ActivationFunctionType
ALU = mybir.AluOpType
bf16 = ml_dtypes.bfloat16

# ---- problem constants (hardcoded) ----
B, NG, F, ED, EG = 1024, 32, 16, 6, 256
H1, C1, C2 = 5, 80, 160
OBS, ACT = 512, 2
NCORES = 8
G = B // NCORES          # 128 graphs per core
NNODES = G * NG          # 4096
NEDGES = G * EG          # 32768
NGRP = NNODES // 128     # 32 groups of 128 nodes (4 graphs)
ETPG = 8                 # edge tiles (of 128) per group (1024 edges)
HC1 = H1 * C1            # 400

_CACHE = {}


def build_nc():
    nc = bacc.Bacc("TRN2", target_bir_lowering=False, debug=False)

    def par(name, shape, dt, out=False):
        return nc.declare_dram_parameter(name, list(shape), dt, isOutput=out)

    d_xT = par("xT", [17, NNODES], BF)            # x.T with ones row 16
    d_ea = par("ea6", [ED, NEDGES], BF)           # edge_attr.T
    d_ST = par("ST", [128, NEDGES], BF)           # src one-hot, node-in-group major
    d_DT = par("DT", [128, NEDGES], BF)           # dst one-hot
    d_Se = par("SeR", [NGRP, 128, ETPG, NG], BF)   # src one-hot edge-major
    d_De = par("DeR", [NGRP, 128, ETPG, NG], BF)
    d_W1 = par("Wlr1b", [17, 832], BF)            # [Wl1.T|a] , [Wr1.T|a] (+bias row)
    d_We1 = par("We1be", [ED, 416], BF)           # [We1.T | We1.T@att]
    d_att1r = par("att1rep", [128, 416], BF)      # attm1 replicated (non-AGS path)
    d_on1 = par("ones1", [1, 128], BF)            # K=1 ones row for bias matmuls
    d_bc1r = par("bc1row", [1, 405], BF)          # bc1 interleaved row (o1 bias)
    d_bc2r = par("bc2row", [1, C2], BF)
    d_eps1 = par("eps1", [1, H1], BF)
    d_eps2 = par("eps2", [1, 1], BF)
    d_W2 = par("Wlr2b", [128, 4, 324], BF)        # chunked [512pad, 322]
    d_We2 = par("We2be", [ED, 162], BF)
    d_att2 = par("att2rep", [128, 162], BF)
    d_wd1 = par("Wd1T", [C2, 32], BF)
    d_wd2 = par("Wd2T", [32, OBS], BF)
    d_wf1 = par("Wf1T", [128, 4, 256], BF)        # [512,256] chunked on k
    d_wf2 = par("Wf2T", [128, 2, 256], BF)
    d_wms = par("WmsT", [128, 2, 4], BF)          # [256, 4] chunked
    d_bd1 = par("bd1", [32, 1], F32)
    d_bd2 = par("bd2c", [128, 4], F32)
    d_bf1 = par("bf1c", [128, 2], F32)
    d_bf2 = par("bf2c", [128, 2], F32)
    d_bms = par("bms", [4, 1], F32)
    d_out = par("out", [4, 128], F32, out=True)

    with tile.TileContext(nc) as tc:
        import contextlib

        ctx = contextlib.ExitStack()
        with ctx:
            cpool = ctx.enter_context(tc.tile_pool(name="const", bufs=1))
            persist = ctx.enter_context(tc.tile_pool(name="persist", bufs=1))
            spool = ctx.enter_context(tc.tile_pool(name="stream", bufs=9))
            wpool = ctx.enter_context(tc.tile_pool(name="work", bufs=5))
            npool = ctx.enter_context(tc.tile_pool(name="small", bufs=5))
            ppool = ctx.enter_context(tc.tile_pool(name="psum", bufs=1, space="PSUM"))
            ppre = ctx.enter_context(tc.tile_pool(name="psumpre", bufs=5, space="PSUM"))
            ppool1 = ctx.enter_context(tc.tile_pool(name="psum1", bufs=2, space="PSUM"))

            # ---- constants to SBUF ----
            def cload(dram, shape, dt):
                t = cpool.tile(list(shape), dt, tag=dram.name + "_c")
                nc.sync.dma_start(out=t[...], in_=dram.ap())
                return t

            s_W1 = cload(d_W1, [17, 832], BF)
            s_We1 = cload(d_We1, [ED, 416], BF)
            s_att1r = cload(d_att1r, [128, 416], BF)
            s_att2x = s_att1r[:, None, 0:410].broadcast_to([128, 2, 410])
            s_on1 = cload(d_on1, [1, 128], BF)
            s_bc1r = cload(d_bc1r, [1, 405], BF)
            s_bc2r = cload(d_bc2r, [1, C2], BF)
            s_eps1 = cload(d_eps1, [1, H1], BF)
            s_eps2 = cload(d_eps2, [1, 1], BF)
            s_W2 = cload(d_W2, [128, 4, 324], BF)
            s_We2 = cload(d_We2, [ED, 162], BF)
            s_att2 = cload(d_att2, [128, 162], BF)
            s_wd1a = cpool.tile([128, 32], BF, tag="wd1a")
            nc.sync.dma_start(out=s_wd1a[...], in_=d_wd1.ap()[0:128, :])
            s_wd1b = cpool.tile([32, 32], BF, tag="wd1b")
            nc.sync.dma_start(out=s_wd1b[...], in_=d_wd1.ap()[128:160, :])
            s_wd2 = cload(d_wd2, [32, OBS], BF)
            s_wf1 = cload(d_wf1, [128, 4, 256], BF)
            s_wf2 = cload(d_wf2, [128, 2, 256], BF)
            s_wms = cload(d_wms, [128, 2, 4], BF)
            s_bd1 = cload(d_bd1, [32, 1], F32)
            s_bd2 = cload(d_bd2, [128, 4], F32)
            s_bf1 = cload(d_bf1, [128, 2], F32)
            s_bf2 = cload(d_bf2, [128, 2], F32)
            s_bm = cpool.tile([2, 1], F32, tag="bm_c")
            nc.sync.dma_start(out=s_bm[...], in_=d_bms.ap()[0:2, :])
            s_bs = cpool.tile([2, 1], F32, tag="bs_c")
            nc.sync.dma_start(out=s_bs[...], in_=d_bms.ap()[2:4, :])

            ident = cpool.tile([128, 128], BF, tag="ident")
            make_identity(nc, ident[...])

            # ---- persistent activations (rotating group-slot windows) ----
            XS, H1S, HTS, X2S, H2S = 4, 3, 3, 4, 3
            xlr = persist.tile([128, XS, 832], BF)      # [xl|a1l|xr|a1r] slots
            h1 = persist.tile([128, H1S, HC1], BF)
            h1T = persist.tile([128, HTS, 4, 128], BF)  # h1 transposed, c-chunked
            xlr2 = persist.tile([128, X2S, 324], BF)
            h2 = persist.tile([128, H2S, C2], BF)
            ego = persist.tile([128, C2], BF)
            egoT0 = persist.tile([128, 128], BF)
            egoT1 = persist.tile([32, 128], BF)
            d1_sb = persist.tile([32, 128], BF)
            d_sb = persist.tile([128, 4, 128], BF)
            f1_sb = persist.tile([128, 2, 128], BF)
            f2_sb = persist.tile([128, 2, 128], BF)
            out_m = persist.tile([2, 128], F32)
            out_s = persist.tile([2, 128], F32)
            ts_sb = persist.tile([2, 128], F32)

            # h1T padding rows: zero tail of chunk 3, ones row 400 (=chunk3 row 32)
            nc.vector.memset(h1T[:, :, 3, :], 0.0)
            nc.vector.memset(h1T[32:64, :, 3, :], 1.0)

            # ---- per-group main loop: software pipeline, l2 lags l1 by one ----
            def s1_dma(grp):
                # layer-1 node transform for this group (phase A, pipelined)
                xs = grp % XS
                xt = spool.tile([17, 128], BF, tag="xt")
                nc.sync.dma_start(out=xt[...], in_=d_xT.ap()[:, grp * 128:(grp + 1) * 128])
                psA = ppre.tile([128, 410], F32, tag="pre", bufs=2, padded_shape=[128, 512])
                psB = ppre.tile([128, 410], F32, tag="pre", bufs=2, padded_shape=[128, 512])
                nc.tensor.matmul(psA[...], xt[...], s_W1[:, 0:410], start=True, stop=True)
                nc.tensor.matmul(psB[...], xt[...], s_W1[:, 416:826], start=True, stop=True)
                nc.scalar.activation(xlr[:, xs, 0:410], psA[...], AF.Copy)
                nc.scalar.activation(xlr[:, xs, 416:826], psB[...], AF.Copy)

                e0 = grp * 1024
                st_t = spool.tile([128, 1024], BF, tag="st")
                dt_t = spool.tile([128, 1024], BF, tag="dt")
                se_t = spool.tile([128, ETPG, NG], BF, tag="se")
                de_t = spool.tile([128, ETPG, NG], BF, tag="de")
                ea1 = spool.tile([ED, 1024], BF, tag="ea")
                nc.sync.dma_start(out=st_t[...], in_=d_ST.ap()[:, e0:e0 + 1024])
                nc.sync.dma_start(out=dt_t[...], in_=d_DT.ap()[:, e0:e0 + 1024])
                nc.sync.dma_start(out=se_t[...], in_=d_Se.ap()[grp])
                nc.sync.dma_start(out=de_t[...], in_=d_De.ap()[grp])
                nc.sync.dma_start(out=ea1[...], in_=d_ea.ap()[:, e0:e0 + 1024])
                score1 = npool.tile([128, ETPG, H1], F32, tag="score", bufs=7)
                return dict(st=st_t, dt=dt_t, se=se_t, de=de_t, ea=ea1,
                            score1=score1)

            def s1_tile(grp, S, t):
                xs = grp % XS
                pre = ppre.tile([128, 410], F32, tag="pre", bufs=2, padded_shape=[128, 512])
                nc.tensor.matmul(pre[...], S["st"][:, t * 128:(t + 1) * 128],
                                 xlr[:, xs, 0:410], start=True, stop=False)
                nc.tensor.matmul(pre[...], S["dt"][:, t * 128:(t + 1) * 128],
                                 xlr[:, xs, 416:826], start=False, stop=False)
                nc.tensor.matmul(pre[...], S["ea"][:, t * 128:(t + 1) * 128],
                                 s_We1[:, 0:410], start=False, stop=True)
                if t % 2 == 0:
                    S["pm2"] = wpool.tile([128, 2, 410], BF, tag="pmr", bufs=4, name="pm2")
                pm2 = S["pm2"]
                nc.scalar.activation(pm2[:, t % 2, :], pre[...], AF.Relu)
                if t % 2 == 1:
                    pma = wpool.tile([128, 2, 410], BF, tag="pma", bufs=4, name="pma")
                    nc.vector.tensor_tensor(pma[...], pm2[...],
                                            s_att2x[...], op=ALU.mult)
                    nc.vector.tensor_reduce(
                        S["score1"][:, t - 1:t + 1, :],
                        pma[...].rearrange("p u (h c) -> p (u h) c", h=H1),
                        axis=mybir.AxisListType.X, op=ALU.add)

            def l1rest(grp, S):
                xs, hs1 = grp % XS, grp % H1S
                se_t, de_t = S["se"], S["de"]
                ex1 = npool.tile([128, ETPG, H1], BF, tag="ex1", bufs=4)
                nc.scalar.activation(ex1[...], S["score1"][...], AF.Exp)
                DN0, RG0 = H1 * NG, H1 * NG + H1
                at1 = ppool1.tile([128, RG0 + ETPG * H1], F32, tag="ats", bufs=1, padded_shape=[128, 512])
                nc.tensor.matmul(at1[:, DN0:RG0], s_on1[...], s_eps1[...],
                                 start=True, stop=False, skip_group_check=True)
                for t in range(ETPG):
                    g = t // 2
                    first = (t % 2 == 0)
                    nc.tensor.matmul(at1[32 * g:32 * (g + 1), DN0:RG0], de_t[:, t, :],
                                     ex1[:, t, :], start=False, stop=not first,
                                     tile_position=(0, 32 * g), skip_group_check=True)
                rec1f = npool.tile([128, H1], F32, tag="rec1f")
                nc.vector.reciprocal(rec1f[...], at1[:, DN0:RG0])
                rec1 = npool.tile([128, H1], BF, tag="rec1")
                nc.vector.tensor_copy(rec1[...], rec1f[...])
                dt_t = S["dt"]
                for t in range(ETPG):
                    c0 = RG0 + t * H1
                    nc.tensor.matmul(at1[:, c0:c0 + H1],
                                     dt_t[:, t * 128:(t + 1) * 128],
                                     rec1[...], start=True, stop=True)
                alf = npool.tile([128, ETPG, H1], BF, tag="alf", bufs=4)
                nc.vector.tensor_mul(alf[...], ex1[...],
                                     at1[:, RG0:].rearrange("p (t h) -> p t h", h=H1))
                for t in range(ETPG):
                    g = t // 2
                    first = (t % 2 == 0)
                    exd = wpool.tile([128, H1, NG], BF, tag="exd", bufs=8)
                    if t >= 4:
                        nc.gpsimd.tensor_tensor(
                            exd[...],
                            de_t[:, t, None, :].broadcast_to([128, H1, NG]),
                            alf[:, t, :, None].broadcast_to([128, H1, NG]),
                            op=ALU.mult)
                    else:
                        nc.vector.scalar_tensor_tensor(
                            exd[...],
                            de_t[:, t, None, :].broadcast_to([128, H1, NG]),
                            0.0,
                            alf[:, t, :, None].broadcast_to([128, H1, NG]),
                            op0=ALU.bypass, op1=ALU.mult)
                    nc.tensor.matmul(at1[32 * g:32 * (g + 1), 0:H1 * NG], se_t[:, t, :],
                                     exd[...], start=first, stop=not first,
                                     tile_position=(0, 32 * g))
                at1_sb = wpool.tile([128, H1 * NG], BF, tag="at1sb", bufs=3)
                nc.scalar.activation(at1_sb[...], at1[:, 0:H1 * NG], AF.Copy)
                o1 = ppool.tile([128, 512], F32, tag="o1", bufs=1)
                nc.tensor.matmul(o1[:, 0:405], s_on1[...], s_bc1r[...],
                                 start=True, stop=False, skip_group_check=True)
                for g in range(4):
                    for h in range(H1):
                        nc.tensor.matmul(
                            o1[32 * g:32 * (g + 1), h * 81:h * 81 + 81],
                            at1_sb[32 * g:32 * (g + 1), h * 32:(h + 1) * 32],
                            xlr[32 * g:32 * (g + 1), xs, h * 82:h * 82 + 81],
                            start=False, stop=True, tile_position=(32 * g, 32 * g),
                            skip_group_check=True)
                ov = o1[:, 0:405].rearrange("p (h c) -> p h c", h=H1)
                nc.scalar.activation(h1[:, hs1, :], ov[:, :, 0:80], AF.Relu)

            def ladder(grp):
                hs1, ht = grp % H1S, grp % HTS
                trp = ppool.tile([128, 4, 128], BF, tag="mt", bufs=1, padded_shape=[128, 8, 128])
                for j in range(4):
                    csz = 128 if j < 3 else 16
                    nc.tensor.transpose(trp[0:csz, j, :], h1[:, hs1, j * 128:j * 128 + csz],
                                        ident[...])
                nc.scalar.activation(h1T[:, ht, 0:3, :], trp[:, 0:3, :], AF.Copy)
                nc.scalar.activation(h1T[0:16, ht, 3, :], trp[0:16, 3, :], AF.Copy)

            def xlr2fn(grp):
                ht, x2 = grp % HTS, grp % X2S
                ps2 = ppool.tile([128, 324], F32, tag="mt", bufs=1, padded_shape=[128, 512])
                for j in range(4):
                    nc.tensor.matmul(ps2[...], h1T[:, ht, j, :],
                                     s_W2[:, j, :], start=(j == 0), stop=(j == 3))
                nc.scalar.activation(xlr2[:, x2, :], ps2[...], AF.Copy)

            def l2pre_tile(grp, S, t):
                x2 = grp % X2S
                pre2 = ppre.tile([128, 162], F32, tag="pre2", bufs=2, padded_shape=[128, 512])
                nc.tensor.matmul(pre2[...], S["st"][:, t * 128:(t + 1) * 128],
                                 xlr2[:, x2, 0:162], start=True, stop=False)
                nc.tensor.matmul(pre2[...], S["dt"][:, t * 128:(t + 1) * 128],
                                 xlr2[:, x2, 162:324], start=False, stop=False)
                nc.tensor.matmul(pre2[...], S["ea"][:, t * 128:(t + 1) * 128],
                                 s_We2[...], start=False, stop=True)
                pma2 = wpool.tile([128, 162], BF, tag="pma2", bufs=3)
                nc.vector.scalar_tensor_tensor(pma2[...], pre2[:, 0:162], 0.0, s_att2[...],
                                               op0=ALU.max, op1=ALU.mult,
                                               accum_out=S["score2"][:, t:t + 1])

            def l2rest(grp, S):
                x2, hs2 = grp % X2S, grp % H2S
                se_t, de_t, dt_t = S["se"], S["de"], S["dt"]
                ex2 = npool.tile([128, ETPG], BF, tag="ex2", bufs=3)
                nc.scalar.activation(ex2[...], S["score2"][...], AF.Exp)
                # at2 layout: [A-norm 0:32 | den 32:33 | rec-gather 33:41]
                at2 = ppre.tile([128, NG + 1 + ETPG], F32, tag="pre2", bufs=2,
                                padded_shape=[128, 512])
                nc.tensor.matmul(at2[:, NG:NG + 1], s_on1[...], s_eps2[...],
                                 start=True, stop=False, skip_group_check=True)
                for t in range(ETPG):
                    g = t // 2
                    first = (t % 2 == 0)
                    nc.tensor.matmul(at2[32 * g:32 * (g + 1), NG:NG + 1], de_t[:, t, :],
                                     ex2[:, t:t + 1], start=False, stop=not first,
                                     tile_position=(0, 32 * g), skip_group_check=True)
                rec2f = npool.tile([128, 1], F32, tag="rec2f")
                nc.vector.reciprocal(rec2f[...], at2[:, NG:NG + 1])
                rec2 = npool.tile([128, 1], BF, tag="rec2")
                nc.vector.tensor_copy(rec2[...], rec2f[...])
                for t in range(ETPG):
                    c0 = NG + 1 + t
                    nc.tensor.matmul(at2[:, c0:c0 + 1],
                                     dt_t[:, t * 128:(t + 1) * 128],
                                     rec2[...], start=True, stop=True)
                alf2 = npool.tile([128, ETPG], BF, tag="alf2", bufs=3)
                nc.vector.tensor_mul(alf2[...], ex2[...], at2[:, NG + 1:])
                exd2 = wpool.tile([128, ETPG, NG], BF, tag="exd2", bufs=3)
                nc.vector.scalar_tensor_tensor(
                    exd2[...], de_t[...], 0.0,
                    alf2[:, :, None].broadcast_to([128, ETPG, NG]),
                    op0=ALU.bypass, op1=ALU.mult)
                for t in range(ETPG):
                    g = t // 2
                    first = (t % 2 == 0)
                    nc.tensor.matmul(at2[32 * g:32 * (g + 1), 0:NG], se_t[:, t, :],
                                     exd2[:, t, :], start=first, stop=not first,
                                     tile_position=(0, 32 * g))
                at2_sb = wpool.tile([128, NG], BF, tag="at2sb", bufs=2)
                nc.scalar.activation(at2_sb[...], at2[:, 0:NG], AF.Copy)
                o2 = ppool.tile([128, C2], F32, tag="o2", bufs=1, padded_shape=[128, 512])
                nc.tensor.matmul(o2[...], s_on1[...], s_bc2r[...],
                                 start=True, stop=False, skip_group_check=True)
                for g in range(4):
                    nc.tensor.matmul(o2[32 * g:32 * (g + 1), :], at2_sb[32 * g:32 * (g + 1), :],
                                     xlr2[32 * g:32 * (g + 1), x2, 0:C2],
                                     start=False, stop=True, tile_position=(32 * g, 32 * g),
                                     skip_group_check=True)
                nc.scalar.activation(h2[:, hs2, :], o2[...], AF.Relu)
                # extract this group's 4 ego rows (graphs 4*grp + a), natural order
                nc.sync.dma_start(out=ego[4 * grp:4 * grp + 4, :],
                                  in_=h2[0:128:32, hs2, :])

            Ss = {}
            for g0 in (0, 1):
                Ss[g0] = s1_dma(g0)
                for t in range(ETPG):
                    s1_tile(g0, Ss[g0], t)
            for g in range(NGRP):
                S = Ss[g]
                S["score2"] = npool.tile([128, ETPG], F32, tag="score2", name="score2")
                l1rest(g, S)
                if g + 2 < NGRP:
                    Ss[g + 2] = s1_dma(g + 2)
                    for t in range(4):
                        s1_tile(g + 2, Ss[g + 2], t)
                ladder(g)
                xlr2fn(g)
                if g + 2 < NGRP:
                    for t in range(4, ETPG):
                        s1_tile(g + 2, Ss[g + 2], t)
                if g >= 2:
                    Sp = Ss[g - 2]
                    for t in range(ETPG):
                        l2pre_tile(g - 2, Sp, t)
                    l2rest(g - 2, Sp)
                    del Ss[g - 2]
            for gl in (NGRP - 2, NGRP - 1):
                Sp = Ss[gl]
                for t in range(ETPG):
                    l2pre_tile(gl, Sp, t)
                l2rest(gl, Sp)
                del Ss[gl]

            tpa = ppool.tile([128, 128], BF, tag="mt", bufs=1, padded_shape=[128, 1024])
            nc.tensor.transpose(tpa[...], ego[:, 0:128], ident[...])
            nc.scalar.activation(egoT0[...], tpa[...], AF.Copy)
            tpb = ppool.tile([32, 128], BF, tag="mt", bufs=1, padded_shape=[128, 1024])
            nc.tensor.transpose(tpb[...], ego[:, 128:160], ident[...])
            nc.scalar.activation(egoT1[...], tpb[...], AF.Copy)

            # ---- head MLP (feature-major, batch=128 on free dim) ----
            d1p = ppool.tile([32, 128], F32, tag="mt", bufs=1, padded_shape=[128, 512])
            nc.tensor.matmul(d1p[...], s_wd1a[...], egoT0[...], start=True, stop=False)
            nc.tensor.matmul(d1p[...], s_wd1b[...], egoT1[...], start=False, stop=True)
            nc.scalar.activation(d1_sb[...], d1p[...], AF.Identity, bias=s_bd1[...])
            for j in range(4):
                dp = ppool.tile([128, 128], F32, tag="mt", bufs=1, padded_shape=[128, 512])
                nc.tensor.matmul(dp[...], s_wd2[:, j * 128:(j + 1) * 128], d1_sb[...],
                                 start=True, stop=True)
                nc.scalar.activation(d_sb[:, j, :], dp[...], AF.Tanh, bias=s_bd2[:, j:j + 1])
            for m in range(2):
                fp = ppool.tile([128, 128], F32, tag="mt", bufs=1, padded_shape=[128, 512])
                for j in range(4):
                    nc.tensor.matmul(fp[...], s_wf1[:, j, m * 128:(m + 1) * 128], d_sb[:, j, :],
                                     start=(j == 0), stop=(j == 3))
                nc.scalar.activation(f1_sb[:, m, :], fp[...], AF.Relu, bias=s_bf1[:, m:m + 1])
            for m in range(2):
                fp2 = ppool.tile([128, 128], F32, tag="mt", bufs=1, padded_shape=[128, 512])
                for j in range(2):
                    nc.tensor.matmul(fp2[...], s_wf2[:, j, m * 128:(m + 1) * 128], f1_sb[:, j, :],
                                     start=(j == 0), stop=(j == 1))
                nc.scalar.activation(f2_sb[:, m, :], fp2[...], AF.Relu, bias=s_bf2[:, m:m + 1])
            msp = ppool.tile([2, 128], F32, tag="mt", bufs=1, padded_shape=[128, 512])
            ssp = ppool.tile([2, 128], F32, tag="mt", bufs=1, padded_shape=[128, 512])
            for j in range(2):
                nc.tensor.matmul(msp[...], s_wms[:, j, 0:2], f2_sb[:, j, :],
                                 start=(j == 0), stop=(j == 1))
            for j in range(2):
                nc.tensor.matmul(ssp[...], s_wms[:, j, 2:4], f2_sb[:, j, :],
                                 start=(j == 0), stop=(j == 1))
            nc.scalar.activation(out_m[...], msp[...], AF.Identity, bias=s_bm[...])
            nc.scalar.activation(ts_sb[...], ssp[...], AF.Tanh, bias=s_bs[...])
            nc.scalar.activation(out_s[...], ts_sb[...], AF.Copy, scale=3.5, bias=-1.5)
            nc.sync.dma_start(out=d_out.ap()[0:2, :], in_=out_m[...])
            nc.sync.dma_start(out=d_out.ap()[2:4, :], in_=out_s[...])

    nc.compile()
    return nc


def _prep_core_inputs(inputs, c):
    """Host-side preprocessing for core c (all free / untimed)."""
    ns = slice(c * NNODES, (c + 1) * NNODES)
    es = slice(c * NEDGES, (c + 1) * NEDGES)
    x = np.asarray(inputs["x"])[ns]                       # [4096, 16]
    ea = np.asarray(inputs["edge_attr"])[es]              # [32768, 6]
    src = np.asarray(inputs["edge_index"])[0, es] - c * NNODES
    dst = np.asarray(inputs["edge_index"])[1, es] - c * NNODES

    e = np.arange(NEDGES)
    ST = np.zeros((128, NEDGES), np.float32)
    DT = np.zeros((128, NEDGES), np.float32)
    ST[src % 128, e] = 1.0
    DT[dst % 128, e] = 1.0
    Se = np.zeros((NEDGES, NG), np.float32)
    De = np.zeros((NEDGES, NG), np.float32)
    Se[e, src % NG] = 1.0
    De[e, dst % NG] = 1.0
    SeR = Se.reshape(NGRP, ETPG, 128, NG).transpose(0, 2, 1, 3)
    DeR = De.reshape(NGRP, ETPG, 128, NG).transpose(0, 2, 1, 3)

    xT = np.concatenate([x.T, np.ones((1, NNODES), np.float32)], 0)  # [17, 4096]

    return {
        "xT": xT.astype(bf16),
        "ea6": ea.T.astype(bf16).copy(),
        "ST": ST.astype(bf16),
        "DT": DT.astype(bf16),
        "SeR": np.ascontiguousarray(SeR).astype(bf16),
        "DeR": np.ascontiguousarray(DeR).astype(bf16),
    }


def _prep_weights(inputs):
    M2 = 4.0      # M/2: term1 offset (cancels in softmax), keeps relu slot positive
    ii = {k: np.asarray(v).astype(np.float32) for k, v in inputs.items()
          if k not in ("x", "edge_index", "edge_attr")}
    att1 = ii["att1"]                                     # [5, 80]

    def inter1(Wt, is_bias_row):
        """Interleave [*, 400] -> [*, 410]: per head [W(80) | ones | att-dot]."""
        out = np.zeros((Wt.shape[0], 410), np.float32)
        for h in range(H1):
            out[:, h * 82:h * 82 + 80] = Wt[:, h * C1:(h + 1) * C1]
            out[:, h * 82 + 81] = Wt[:, h * C1:(h + 1) * C1] @ att1[h]
            if is_bias_row:
                out[-1, h * 82 + 80] = 1.0      # ones col (agg reads l-side only)
                out[-1, h * 82 + 81] += M2
        return out

    Wl = np.concatenate([ii["Wl1"].T, ii["bl1"][None, :]], 0)   # [17, 400]
    Wr = np.concatenate([ii["Wr1"].T, ii["br1"][None, :]], 0)
    W1 = np.zeros((17, 832), np.float32)
    W1[:, 0:410] = inter1(Wl, True)
    W1[:, 416:826] = inter1(Wr, True)
    We1 = np.zeros((ED, 416), np.float32)
    We1[:, 0:410] = inter1(ii["We1"].T, False)       # [6, 410]
    attm1 = np.zeros(410, np.float32)
    for h in range(H1):
        attm1[h * 82:h * 82 + 80] = 0.8 * att1[h]
        attm1[h * 82 + 81] = 0.2

        att2 = ii["att2"].reshape(-1)                         # [160]
    W2 = np.zeros((512, 324), np.float32)
    W2[0:400, 0:160] = ii["Wl2"].T
    W2[0:400, 161] = ii["Wl2"].T @ att2
    W2[0:400, 162:322] = ii["Wr2"].T
    W2[0:400, 323] = ii["Wr2"].T @ att2
    W2[416, 0:160] = ii["bl2"]
    W2[416, 160] = 1.0                               # ones col (l side)
    W2[416, 161] = ii["bl2"] @ att2 + M2
    W2[416, 162:322] = ii["br2"]
    W2[416, 323] = ii["br2"] @ att2 + M2
    We2 = np.zeros((ED, 162), np.float32)
    We2[:, 0:160] = ii["We2"].T
    We2[:, 161] = ii["We2"].T @ att2
    attm2 = np.zeros(162, np.float32)
    attm2[0:160] = 0.8 * att2
    attm2[161] = 0.2

    Wf1 = ii["Wf1"].T.reshape(4, 128, 256).copy()         # [512,256] -> k-chunks
    Wf2 = ii["Wf2"].T.reshape(2, 128, 256).copy()
    Wms = np.concatenate([ii["Wm"].T, ii["Ws"].T], 1)     # [256, 4]
    Wmsr = Wms.reshape(2, 128, 4).copy()

    attm1p = np.zeros(416, np.float32)
    attm1p[0:410] = attm1
    bc1row = np.zeros((1, 405), np.float32)
    for h in range(H1):
        bc1row[0, h * 81:h * 81 + 80] = ii["bc1"][h * C1:(h + 1) * C1]

    return {
        "Wlr1b": W1.astype(bf16),
        "We1be": We1.astype(bf16),
        "att1rep": np.broadcast_to(attm1p, (128, 416)).astype(bf16).copy(),
        "ones1": np.ones((1, 128), np.float32).astype(bf16),
        "bc1row": bc1row.astype(bf16),
        "bc2row": ii["bc2"][None, :].astype(bf16).copy(),
        "eps1": np.full((1, H1), 1e-16, np.float32).astype(bf16),
        "eps2": np.full((1, 1), 1e-16, np.float32).astype(bf16),
        "Wlr2b": W2.reshape(4, 128, 324).transpose(1, 0, 2).astype(bf16).copy(),
        "We2be": We2.astype(bf16),
        "att2rep": np.broadcast_to(attm2, (128, 162)).astype(bf16).copy(),
        "Wd1T": ii["Wd1"].T.astype(bf16).copy(),
        "Wd2T": ii["Wd2"].T.astype(bf16).copy(),
        "Wf1T": np.transpose(Wf1, (1, 0, 2)).astype(bf16).copy(),
        "Wf2T": np.transpose(Wf2, (1, 0, 2)).astype(bf16).copy(),
        "WmsT": np.transpose(Wmsr, (1, 0, 2)).astype(bf16).copy(),
        "bd1": ii["bd1"][:, None].astype(np.float32).copy(),
        "bd2c": ii["bd2"].reshape(4, 128).T.astype(np.float32).copy(),
        "bf1c": ii["bf1"].reshape(2, 128).T.astype(np.float32).copy(),
        "bf2c": ii["bf2"].reshape(2, 128).T.astype(np.float32).copy(),
        "bms": np.concatenate([ii["bm"], ii["bs"]])[:, None].astype(np.float32).copy(),
    }


LAST = {}


def kernel(**inputs):
    if "nc" not in _CACHE:
        _CACHE["nc"] = build_nc()
    nc = _CACHE["nc"]

    wts = _prep_weights(inputs)
    in_maps = []
    for c in range(NCORES):
        m = dict(wts)
        m.update(_prep_core_inputs(inputs, c))
        in_maps.append(m)

    res = run_bass_kernel_spmd(nc, in_maps, core_ids=list(range(NCORES)),
                               trace=LAST.get("trace", False))
    LAST["exec_time_ns"] = res.exec_time_ns
    LAST["res"] = res

    mean = np.zeros((B, ACT), np.float32)
    logstd = np.zeros((B, ACT), np.float32)
    p = np.arange(128)
    gperm = p                             # partition p holds graph p
    for c in range(NCORES):
        o = res.results[c]["out"]          # [4, 128]
        mean[c * G + gperm, :] = o[0:2, :].T
        logstd[c * G + gperm, :] = o[2:4, :].T
    return mean, logstd

